# revision 8
# baseline (speedup 1.0000x reference)
"""Trainium2 Bass kernel for the capsule-routing layer (nn_Caps_Layer).

Full inputs: x [32, 512, 768] f32, W [1, 768, 512] f32.
Output: [32, 16, 32] f32.

Strategy: data-parallel over batch across 8 NeuronCores (4 batches/core),
inputs converted to bf16 on the host (halves the HBM traffic; rel-err
budget 2e-2 >> bf16's ~5e-3).

Per core the routing loop is algebraically factored so u_hat [S, N*C]
is never materialized:
    iter0:   m0[(nc)]   = xsum @ W             (xsum = col-sum of x)
    V[d,n]   = sum_c W[d,(n c)] * mnorm[n,c]   (Wt-chunk @ Mblk, ap=16)
    b[s,n]   = x @ V                           (xT-chunk @ V,     ap=16)
    c        = softmax_n(b)
    G[n,d]   = c^T @ x                         (x-chunk @ c,      ap=16)
    m[(nc)]  = diag_n(W^T G)                   (W-chunk @ G^T,    ap=16)
    squash: inv = exp(-0.5 ln(|m|^2 + eps))    (one ACT table: exp+ln)
All routing matmuls keep the tiny capsule dim (16) as the moving side, so
PE streaming cost is ~16 cycles/matmul; the only large PE work is the
x-transposes (needed for the d-major contraction in b = x @ V).
"""
import numpy as np
import concourse.bass as bass
import concourse.mybir as mybir
import concourse.tile as tile
from concourse import bacc
from concourse.bass import ts, ds
from concourse.bass_utils import run_bass_kernel_spmd

F32 = mybir.dt.float32
U32 = mybir.dt.uint32
BF16 = mybir.dt.bfloat16
AF = mybir.ActivationFunctionType
AX = mybir.AxisListType
OP = mybir.AluOpType

NCORES = 8
B, S, D = 32, 512, 768
N, C = 16, 32
NC = N * C            # 512
BL = B // NCORES      # 4 batches per core
EPS = 1e-7
SCN = S // 128        # 4 s-chunks
DCN = D // 128        # 6 d-chunks
KCN = NC // 128       # 4 nc-chunks
ROUTINGS = 3
PE_WARM = 600  # dummy PE transposes chained to hold the p-state at 2.4 GHz

# const tile column layout (all bf16)
CID = 0               # [128, 128] identity (PE transposes)
CMASK = 128           # [128, 256] diag mask[(nl,c), (b,kc,n)] = (n == 4*kc+nl)
CSEL = 384            # [128, 4]   sel[p, j] = (p//32 == j)
CONE = 388            # [128, 1]   ones
CSELT = 392           # rows 0:4, cols 392:520: selt[j, p] = (p//32 == j)
CONW = 520


def _build_module():
    nc = bacc.Bacc("TRN2", target_bir_lowering=False, num_devices=NCORES)
    X = nc.dram_tensor("x", [BL, S, D], BF16, kind="ExternalInput")
    W = nc.dram_tensor("w", [D, NC], BF16, kind="ExternalInput")
    CON = nc.dram_tensor("consts", [128, CONW], BF16, kind="ExternalInput")
    OUT = nc.dram_tensor("out", [BL, N, C], F32, kind="ExternalOutput")

    cp_flip = [0]

    with tile.TileContext(nc) as tc:
        with (
            tc.tile_pool(name="const", bufs=1) as pc,
            tc.tile_pool(name="rt", bufs=2) as prt,
            tc.tile_pool(name="pmm", bufs=1, space="PSUM") as pmm,
            tc.tile_pool(name="ptr", bufs=3, space="PSUM") as ptr,
        ):
            def cp(dst, src):
                # PSUM->SBUF evacuations: 5:3 DVE:ACT (DVE copies are ~1.6x
                # faster per element with packed bf16)
                r = cp_flip[0] % 8
                if r in (0, 2, 3, 5, 7):
                    nc.vector.tensor_copy(dst, src)
                else:
                    nc.scalar.copy(dst, src)
                cp_flip[0] += 1

            # ---- persistent tiles ----
            con = pc.tile([128, CONW], BF16, tag="con")
            wsb = pc.tile([128, DCN, NC], BF16, tag="w")
            wtsb = pc.tile([128, KCN, D], BF16, tag="wt")
            xsumb = pc.tile([128, BL * DCN], BF16, tag="xsum")
            epst = pc.tile([128, 1], F32, tag="eps")
            nc.vector.memset(epst[:], EPS)
            magict = pc.tile([128, 16], U32, tag="magic")
            nc.vector.memset(magict[:], 0x5F3759DF)

            def prefetch_act(func):
                # dummy [1,1] activation hoists the ACT table load early
                dum = prt.tile([1, 1], F32, tag="dum")
                nc.scalar.activation(dum[:], epst[0:1, :], func)

            # consts ride the ACT queue; x batches + W ride the SP queue.
            # W sits between x1 and x2 so WT transposes clear the PE early;
            # the last batch arrives in s-chunks so stage A can track it.
            nc.scalar.dma_start(con[:], CON[:, :])
            xbs = [pc.tile([128, SCN, D], BF16, tag=f"xb{b}", name=f"xb_{b}")
                   for b in range(BL)]
            nc.sync.dma_start(
                xbs[0][:], X[0, :, :].rearrange("(sc p) d -> p sc d", p=128)
            )
            nc.sync.dma_start(
                wsb[:], W[:, :].rearrange("(dc p) n -> p dc n", p=128)
            )
            for b in (1, 2):
                nc.sync.dma_start(
                    xbs[b][:],
                    X[b, :, :].rearrange("(sc p) d -> p sc d", p=128),
                )
            for sc in range(SCN):
                nc.sync.dma_start(
                    xbs[3][:, sc, :], X[3, ds(sc * 128, 128), :]
                )
            prefetch_act(AF.Exp)

            ident = con[:, CID:CID + 128]

            # ---- stage A: xT + xsum per batch; WT between b2 and b3 so
            # the W transposes fill the DMA wait for the last batch ----
            pxs = pmm.tile([128, BL * DCN], F32, tag="sm")
            xts = [pc.tile([128, DCN, S], BF16, tag=f"xt{b}", name=f"xt_{b}")
                   for b in range(BL)]

            def stage_a(b):
                xb = xbs[b]
                xt = xts[b]
                for dc in range(DCN):
                    pxt = ptr.tile([128, S], BF16, tag="tr")
                    for sc in range(SCN):
                        nc.tensor.transpose(
                            pxt[:, ts(sc, 128)],
                            xb[:, sc, ds(dc * 128, 128)],
                            ident,
                        )
                    cp(xt[:, dc, :], pxt[:])
                    for sc in range(SCN):
                        nc.tensor.matmul(
                            pxs[:, ds(b * DCN + dc, 1)],
                            xb[:, sc, ds(dc * 128, 128)],
                            con[:, CONE:CONE + 1],
                            start=(sc == 0),
                            stop=(sc == SCN - 1),
                        )
                cp(xsumb[:, ds(b * DCN, DCN)], pxs[:, ds(b * DCN, DCN)])

            for b in range(3):
                stage_a(b)
            for kc in range(KCN):
                ptw = ptr.tile([128, S], BF16, tag="tr")
                for dc in range(DCN):
                    half = dc // 4
                    if dc % 4 == 0 and half == 1:
                        cp(wtsb[:, kc, 0:512], ptw[:])
                        ptw = ptr.tile([128, S], BF16, tag="tr")
                    nc.tensor.transpose(
                        ptw[:, ts(dc % 4, 128)],
                        wsb[:, dc, ds(kc * 128, 128)],
                        ident,
                    )
                cp(wtsb[:, kc, 512:768], ptw[:, 0:256])
            stage_a(3)

            # PE warmer: a serial chain of dummy transposes keeps the PE
            # p-state ramped through the routing phase so the small matmul
            # groups run at full clock.
            if PE_WARM:
                pwm = pmm.tile([128, 128], BF16, tag="warm")
                for _ in range(PE_WARM):
                    nc.tensor.transpose(pwm[:], con[:, 0:128], ident)

            # ---- routing ----
            maskr = con[:, CMASK:CMASK + BL * KCN * N]

            def squash(pot, src_cols, it):
                """pot: psum [128, (b kc[ n])] -> returns mnorm tile.
                src_cols=1 for iter0 (pot is [128, (b kc)] = m directly)."""
                small = prt.tile([128, 16], F32, tag="m", name=f"m{it}")
                if src_cols == 1:
                    nc.vector.tensor_copy(small[:], pot[:])
                    m = small
                else:
                    pm = prt.tile([128, BL * KCN * N], F32, tag="pm")
                    nc.vector.tensor_mul(pm[:], pot[:], maskr)
                    nc.vector.tensor_reduce(
                        small[:],
                        pm[:].rearrange("p (g n) -> p g n", g=BL * KCN),
                        axis=AX.X,
                        op=OP.add,
                    )
                    m = small
                sq = prt.tile([128, 16], BF16, tag="sq", name=f"sq{it}")
                nc.vector.tensor_mul(sq[:], m[:], m[:])
                pnsq = pmm.tile([128, 16], F32, tag="sm", name=f"nsq{it}")
                nc.tensor.matmul(
                    pnsq[0:4, :],
                    con[:, CSEL:CSEL + 4],
                    sq[:],
                    start=True,
                    stop=True,
                )
                # rsqrt on DVE only (bit trick + 1 Newton step); keeps the
                # ACT table pinned to Exp for the whole kernel. nsq is
                # O(10..100) here so the reference's +eps is a no-op in bf16.
                y0u = prt.tile([4, 16], U32, tag="y0u", name=f"y0u{it}")
                nc.vector.tensor_scalar(
                    y0u[:], pnsq[0:4, :].bitcast(U32), 1, None,
                    OP.logical_shift_right,
                )
                nc.vector.tensor_sub(y0u[:], magict[0:4, :], y0u[:])
                y0f = y0u[:].bitcast(F32)
                t1 = prt.tile([4, 16], F32, tag="nt1", name=f"nt1{it}")
                nc.vector.tensor_mul(t1[:], pnsq[0:4, :], y0f)
                nc.vector.tensor_mul(t1[:], t1[:], y0f)
                nc.vector.tensor_scalar(t1[:], t1[:], -0.5, 1.5, OP.mult,
                                        OP.add)
                rsq = prt.tile([4, 16], BF16, tag="rsq", name=f"rsq{it}")
                nc.vector.tensor_mul(rsq[:], y0f, t1[:])
                pinv = pmm.tile([128, 16], F32, tag="sm2", name=f"pinv{it}")
                nc.tensor.matmul(
                    pinv[:],
                    con[0:4, CSELT:CSELT + 128],
                    rsq[:],
                    start=True,
                    stop=True,
                )
                dt = F32 if it == ROUTINGS - 1 else BF16
                mnorm = prt.tile([128, 16], dt, tag=f"mn{it % 2}",
                                 name=f"mn{it}")
                nc.vector.tensor_mul(mnorm[:], m[:], pinv[:])
                return mnorm

            def v_and_b(mnorm, it):
                """Mblk scatter -> V -> b (psum) for the next iteration."""
                mblk = prt.tile([128, BL * KCN * N], BF16, tag="mblk",
                                name=f"mblk{it}")
                mn_bc = bass.AP(
                    tensor=mnorm.tensor,
                    offset=mnorm.offset,
                    ap=[mnorm.ap[0], [KCN, BL], [1, KCN], [0, N]],
                )
                nc.vector.tensor_mul(
                    mblk[:].rearrange("p (b k n) -> p b k n", b=BL, k=KCN),
                    mn_bc,
                    maskr.rearrange("p (b k n) -> p b k n", b=BL, k=KCN),
                )
                pv = pmm.tile([128, BL * DCN * N], F32, tag="big")
                for b in range(BL):
                    for dc in range(DCN):
                        for kc in range(KCN):
                            nc.tensor.matmul(
                                pv[:, ds((b * DCN + dc) * N, N)],
                                wtsb[:, kc, ds(dc * 128, 128)],
                                mblk[:, ds((b * KCN + kc) * N, N)],
                                start=(kc == 0),
                                stop=(kc == KCN - 1),
                            )
                vsb = prt.tile([128, BL * DCN * N], BF16, tag="vsb")
                nc.scalar.copy(vsb[:], pv[:])
                pb = pmm.tile([128, BL * SCN * N], F32, tag="seq")
                for b in range(BL):
                    for sc in range(SCN):
                        for dc in range(DCN):
                            nc.tensor.matmul(
                                pb[:, ds((b * SCN + sc) * N, N)],
                                xts[b][:, dc, ds(sc * 128, 128)],
                                vsb[:, ds((b * DCN + dc) * N, N)],
                                start=(dc == 0),
                                stop=(dc == DCN - 1),
                            )
                return pb

            # iter 0: uniform routing weights -> m0 = xsum @ W (diag blocks)
            pot0 = pmm.tile([128, BL * KCN], F32, tag="seq")
            for b in range(BL):
                for kc in range(KCN):
                    for dc in range(DCN):
                        nc.tensor.matmul(
                            pot0[:, ds(b * KCN + kc, 1)],
                            wsb[:, dc, ds(kc * 128, 128)],
                            xsumb[:, ds(b * DCN + dc, 1)],
                            start=(dc == 0),
                            stop=(dc == DCN - 1),
                        )
            mnorm = squash(pot0, 1, 0)
            pb = v_and_b(mnorm, 0)

            for it in range(1, ROUTINGS):
                # softmax over n
                expb = prt.tile([128, BL * SCN * N], F32, tag="expb",
                                name=f"expb{it}")
                nc.scalar.activation(expb[:], pb[:], AF.Exp)
                zsum = prt.tile([128, BL * SCN], F32, tag="zsum",
                                name=f"zsum{it}")
                nc.vector.tensor_reduce(
                    zsum[:],
                    expb[:].rearrange("p (g n) -> p g n", g=BL * SCN),
                    axis=AX.X,
                    op=OP.add,
                )
                zrec = prt.tile([128, BL * SCN], F32, tag="zrec",
                                name=f"zrec{it}")
                nc.vector.reciprocal(zrec[:], zsum[:])
                cw = prt.tile([128, BL * SCN * N], BF16, tag="cw",
                              name=f"cw{it}")
                zr_bc = bass.AP(
                    tensor=zrec.tensor,
                    offset=zrec.offset,
                    ap=[zrec.ap[0], [1, BL * SCN], [0, N]],
                )
                nc.vector.tensor_mul(
                    cw[:].rearrange("p (g n) -> p g n", g=BL * SCN),
                    expb[:].rearrange("p (g n) -> p g n", g=BL * SCN),
                    zr_bc,
                )
                # G^T[d, n] per (b, dc)
                pg = pmm.tile([128, BL * DCN * N], F32, tag="big",
                              name=f"gp{it}")
                for b in range(BL):
                    for dc in range(DCN):
                        for sc in range(SCN):
                            nc.tensor.matmul(
                                pg[:, ds((b * DCN + dc) * N, N)],
                                xbs[b][:, sc, ds(dc * 128, 128)],
                                cw[:, ds((b * SCN + sc) * N, N)],
                                start=(sc == 0),
                                stop=(sc == SCN - 1),
                            )
                gsb = prt.tile([128, BL * DCN * N], BF16, tag="gsb",
                               name=f"gsb{it}")
                nc.scalar.copy(gsb[:], pg[:])
                # outT[(nc), n] per (b, kc)
                pot = pmm.tile([128, BL * KCN * N], F32, tag="seq",
                               name=f"potp{it}")
                for b in range(BL):
                    for kc in range(KCN):
                        for dc in range(DCN):
                            nc.tensor.matmul(
                                pot[:, ds((b * KCN + kc) * N, N)],
                                wsb[:, dc, ds(kc * 128, 128)],
                                gsb[:, ds((b * DCN + dc) * N, N)],
                                start=(dc == 0),
                                stop=(dc == DCN - 1),
                            )
                mnorm = squash(pot, N, it)
                if it < ROUTINGS - 1:
                    pb = v_and_b(mnorm, it)

            # final output: mnorm [128=(nl,c), (b kc)] f32 -> OUT[b, n, c]
            nc.sync.dma_start(
                OUT.rearrange("b (kc nl) c -> (nl c) (b kc)", kc=KCN, nl=4),
                mnorm[:],
            )

    nc.compile()
    return nc


def _make_consts():
    import ml_dtypes
    con = np.zeros((128, CONW), dtype=np.float32)
    con[:, CID:CID + 128] = np.eye(128, dtype=np.float32)
    p = np.arange(128)
    for b in range(BL):
        for kc in range(KCN):
            for n in range(N):
                con[:, CMASK + (b * KCN + kc) * N + n] = (n == 4 * kc + p // 32)
    for j in range(4):
        con[:, CSEL + j] = (p // 32 == j)
    con[:, CONE] = 1.0
    for j in range(4):
        con[j, CSELT:CSELT + 128] = (p // 32 == j)
    return con.astype(ml_dtypes.bfloat16)


_NC_CACHE = []


def kernel(x: np.ndarray, W: np.ndarray) -> np.ndarray:
    import ml_dtypes
    assert x.shape == (B, S, D) and W.shape == (1, D, NC)
    if not _NC_CACHE:
        _NC_CACHE.append(_build_module())
    nc = _NC_CACHE[0]
    con = _make_consts()
    w2 = np.ascontiguousarray(W[0]).astype(ml_dtypes.bfloat16)
    xb = x.astype(ml_dtypes.bfloat16)
    in_maps = []
    for i in range(NCORES):
        m = {
            "x": np.ascontiguousarray(xb[i * BL:(i + 1) * BL]),
            "w": w2,
            "consts": con,
        }
        in_maps.append(m)
    res = run_bass_kernel_spmd(nc, in_maps, list(range(NCORES)))
    out = np.concatenate([res.results[i]["out"] for i in range(NCORES)], axis=0)
    return out.astype(np.float32)


# revision 9
# speedup vs baseline: 1.6769x; 1.6769x over previous
"""Trainium2 Bass kernel for the capsule-routing layer (nn_Caps_Layer).

Full inputs: x [32, 512, 768] f32, W [1, 768, 512] f32.
Output: [32, 16, 32] f32.

Strategy: data-parallel over batch across 8 NeuronCores (4 batches/core),
inputs converted to bf16 on the host (halves the HBM traffic; rel-err
budget 2e-2 >> bf16's ~5e-3).

Per core the routing loop is algebraically factored so u_hat [S, N*C]
is never materialized:
    iter0:   m0[(nc)]   = xsum @ W             (xsum = col-sum of x)
    V[d,n]   = sum_c W[d,(n c)] * mnorm[n,c]   (Wt-chunk @ Mblk, ap=16)
    b[s,n]   = x @ V                           (xT-chunk @ V,     ap=16)
    c        = softmax_n(b)
    G[n,d]   = c^T @ x                         (x-chunk @ c,      ap=16)
    m[(nc)]  = diag_n(W^T G)                   (W-chunk @ G^T,    ap=16)
    squash: inv = exp(-0.5 ln(|m|^2 + eps))    (one ACT table: exp+ln)
All routing matmuls keep the tiny capsule dim (16) as the moving side, so
PE streaming cost is ~16 cycles/matmul; the only large PE work is the
x-transposes (needed for the d-major contraction in b = x @ V).
"""
import numpy as np
import concourse.bass as bass
import concourse.mybir as mybir
import concourse.tile as tile
from concourse import bacc
from concourse.bass import ts, ds
from concourse.bass_utils import run_bass_kernel_spmd

F32 = mybir.dt.float32
U32 = mybir.dt.uint32
BF16 = mybir.dt.bfloat16
AF = mybir.ActivationFunctionType
AX = mybir.AxisListType
OP = mybir.AluOpType

NCORES = 8
B, S, D = 32, 512, 768
N, C = 16, 32
NC = N * C            # 512
BL = B // NCORES      # 4 batches per core
EPS = 1e-7
SCN = S // 128        # 4 s-chunks
DCN = D // 128        # 6 d-chunks
KCN = NC // 128       # 4 nc-chunks
ROUTINGS = 3
PE_WARM = 0   # dummy PE transposes chained to hold the p-state at 2.4 GHz

# const tile column layout (all bf16)
CID = 0               # [128, 128] identity (PE transposes)
CMASK = 128           # [128, 256] diag mask[(nl,c), (b,kc,n)] = (n == 4*kc+nl)
CSEL = 384            # [128, 4]   sel[p, j] = (p//32 == j)
CONE = 388            # [128, 1]   ones
CSELT = 392           # rows 0:4, cols 392:520: selt[j, p] = (p//32 == j)
CONW = 520


def _build_module():
    nc = bacc.Bacc("TRN2", target_bir_lowering=False, num_devices=NCORES)
    X = nc.dram_tensor("x", [BL, S, D], BF16, kind="ExternalInput")
    W = nc.dram_tensor("w", [D, NC], BF16, kind="ExternalInput")
    CON = nc.dram_tensor("consts", [128, CONW], BF16, kind="ExternalInput")
    OUT = nc.dram_tensor("out", [BL, N, C], F32, kind="ExternalOutput")

    cp_flip = [0]

    with tile.TileContext(nc) as tc:
        with (
            tc.tile_pool(name="const", bufs=1) as pc,
            tc.tile_pool(name="rt", bufs=2) as prt,
            tc.tile_pool(name="pmm", bufs=1, space="PSUM") as pmm,
            tc.tile_pool(name="ptr", bufs=3, space="PSUM") as ptr,
        ):
            def cp(dst, src):
                # PSUM->SBUF evacuations: 5:3 DVE:ACT (DVE copies are ~1.6x
                # faster per element with packed bf16)
                r = cp_flip[0] % 8
                if r in (0, 2, 3, 5, 7):
                    nc.vector.tensor_copy(dst, src)
                else:
                    nc.scalar.copy(dst, src)
                cp_flip[0] += 1

            # ---- persistent tiles ----
            con = pc.tile([128, CONW], BF16, tag="con")
            wsb = pc.tile([128, DCN, NC], BF16, tag="w")
            wtsb = pc.tile([128, KCN, D], BF16, tag="wt")
            xsumb = pc.tile([128, BL * DCN], BF16, tag="xsum")
            epst = pc.tile([128, 1], F32, tag="eps")
            nc.vector.memset(epst[:], EPS)
            magict = pc.tile([128, 16], U32, tag="magic")
            nc.vector.memset(magict[:], 0x5F3759DF)

            def prefetch_act(func):
                # dummy [1,1] activation hoists the ACT table load early
                dum = prt.tile([1, 1], F32, tag="dum")
                nc.scalar.activation(dum[:], epst[0:1, :], func)

            # consts ride the ACT queue; x batches + W ride the SP queue.
            # W sits between x1 and x2 so WT transposes clear the PE early;
            # the last batch arrives in s-chunks so stage A can track it.
            nc.scalar.dma_start(con[:], CON[:, :])
            xbs = [pc.tile([128, SCN, D], BF16, tag=f"xb{b}", name=f"xb_{b}")
                   for b in range(BL)]
            nc.sync.dma_start(
                xbs[0][:], X[0, :, :].rearrange("(sc p) d -> p sc d", p=128)
            )
            nc.sync.dma_start(
                wsb[:], W[:, :].rearrange("(dc p) n -> p dc n", p=128)
            )
            for b in (1, 2):
                nc.sync.dma_start(
                    xbs[b][:],
                    X[b, :, :].rearrange("(sc p) d -> p sc d", p=128),
                )
            for sc in range(SCN):
                nc.sync.dma_start(
                    xbs[3][:, sc, :], X[3, ds(sc * 128, 128), :]
                )
            prefetch_act(AF.Exp)

            ident = con[:, CID:CID + 128]

            # ---- stage A: xT + xsum per batch; WT between b2 and b3 so
            # the W transposes fill the DMA wait for the last batch ----
            pxs = pmm.tile([128, BL * DCN], F32, tag="sm")
            xts = [pc.tile([128, DCN, S], BF16, tag=f"xt{b}", name=f"xt_{b}")
                   for b in range(BL)]

            def stage_a(b):
                xb = xbs[b]
                xt = xts[b]
                for dc in range(DCN):
                    pxt = ptr.tile([128, S], BF16, tag="tr")
                    for sc in range(SCN):
                        nc.tensor.transpose(
                            pxt[:, ts(sc, 128)],
                            xb[:, sc, ds(dc * 128, 128)],
                            ident,
                        )
                    cp(xt[:, dc, :], pxt[:])
                    for sc in range(SCN):
                        nc.tensor.matmul(
                            pxs[:, ds(b * DCN + dc, 1)],
                            xb[:, sc, ds(dc * 128, 128)],
                            con[:, CONE:CONE + 1],
                            start=(sc == 0),
                            stop=(sc == SCN - 1),
                        )
                cp(xsumb[:, ds(b * DCN, DCN)], pxs[:, ds(b * DCN, DCN)])

            for b in range(3):
                stage_a(b)
            for kc in range(KCN):
                ptw = ptr.tile([128, S], BF16, tag="tr")
                for dc in range(DCN):
                    half = dc // 4
                    if dc % 4 == 0 and half == 1:
                        cp(wtsb[:, kc, 0:512], ptw[:])
                        ptw = ptr.tile([128, S], BF16, tag="tr")
                    nc.tensor.transpose(
                        ptw[:, ts(dc % 4, 128)],
                        wsb[:, dc, ds(kc * 128, 128)],
                        ident,
                    )
                cp(wtsb[:, kc, 512:768], ptw[:, 0:256])
            stage_a(3)

            # PE warmer: a serial chain of dummy transposes keeps the PE
            # p-state ramped through the routing phase so the small matmul
            # groups run at full clock.
            if PE_WARM:
                pwm = pmm.tile([128, 128], BF16, tag="warm")
                for _ in range(PE_WARM):
                    nc.tensor.transpose(pwm[:], con[:, 0:128], ident)

            # ---- routing ----
            maskr = con[:, CMASK:CMASK + BL * KCN * N]

            def squash(pot, src_cols, it):
                """pot: psum [128, (b kc[ n])] -> returns mnorm tile.
                src_cols=1 for iter0 (pot is [128, (b kc)] = m directly)."""
                small = prt.tile([128, 16], F32, tag="m", name=f"m{it}")
                if src_cols == 1:
                    nc.vector.tensor_copy(small[:], pot[:])
                    m = small
                else:
                    pm = prt.tile([128, BL * KCN * N], F32, tag="pm")
                    nc.vector.tensor_mul(pm[:], pot[:], maskr)
                    nc.vector.tensor_reduce(
                        small[:],
                        pm[:].rearrange("p (g n) -> p g n", g=BL * KCN),
                        axis=AX.X,
                        op=OP.add,
                    )
                    m = small
                sq = prt.tile([128, 16], BF16, tag="sq", name=f"sq{it}")
                nc.vector.tensor_mul(sq[:], m[:], m[:])
                pnsq = pmm.tile([128, 16], F32, tag="sm", name=f"nsq{it}")
                nc.tensor.matmul(
                    pnsq[0:4, :],
                    con[:, CSEL:CSEL + 4],
                    sq[:],
                    start=True,
                    stop=True,
                )
                # rsqrt on DVE only (bit trick + 1 Newton step); keeps the
                # ACT table pinned to Exp for the whole kernel. nsq is
                # O(10..100) here so the reference's +eps is a no-op in bf16.
                y0u = prt.tile([4, 16], U32, tag="y0u", name=f"y0u{it}")
                nc.vector.tensor_scalar(
                    y0u[:], pnsq[0:4, :].bitcast(U32), 1, None,
                    OP.logical_shift_right,
                )
                nc.vector.tensor_sub(y0u[:], magict[0:4, :], y0u[:])
                y0f = y0u[:].bitcast(F32)
                t1 = prt.tile([4, 16], F32, tag="nt1", name=f"nt1{it}")
                nc.vector.tensor_mul(t1[:], pnsq[0:4, :], y0f)
                nc.vector.tensor_mul(t1[:], t1[:], y0f)
                nc.vector.tensor_scalar(t1[:], t1[:], -0.5, 1.5, OP.mult,
                                        OP.add)
                rsq = prt.tile([4, 16], BF16, tag="rsq", name=f"rsq{it}")
                nc.vector.tensor_mul(rsq[:], y0f, t1[:])
                pinv = pmm.tile([128, 16], F32, tag="sm2", name=f"pinv{it}")
                nc.tensor.matmul(
                    pinv[:],
                    con[0:4, CSELT:CSELT + 128],
                    rsq[:],
                    start=True,
                    stop=True,
                )
                dt = F32 if it == ROUTINGS - 1 else BF16
                mnorm = prt.tile([128, 16], dt, tag=f"mn{it % 2}",
                                 name=f"mn{it}")
                nc.vector.tensor_mul(mnorm[:], m[:], pinv[:])
                return mnorm

            def v_and_b(mnorm, it):
                """Mblk scatter -> V -> b (psum) for the next iteration."""
                mblk = prt.tile([128, BL * KCN * N], BF16, tag="mblk",
                                name=f"mblk{it}")
                mn_bc = bass.AP(
                    tensor=mnorm.tensor,
                    offset=mnorm.offset,
                    ap=[mnorm.ap[0], [KCN, BL], [1, KCN], [0, N]],
                )
                nc.vector.tensor_mul(
                    mblk[:].rearrange("p (b k n) -> p b k n", b=BL, k=KCN),
                    mn_bc,
                    maskr.rearrange("p (b k n) -> p b k n", b=BL, k=KCN),
                )
                pv = pmm.tile([128, BL * DCN * N], F32, tag="big")
                for b in range(BL):
                    for dc in range(DCN):
                        for kc in range(KCN):
                            nc.tensor.matmul(
                                pv[:, ds((b * DCN + dc) * N, N)],
                                wtsb[:, kc, ds(dc * 128, 128)],
                                mblk[:, ds((b * KCN + kc) * N, N)],
                                start=(kc == 0),
                                stop=(kc == KCN - 1),
                            )
                vsb = prt.tile([128, BL * DCN * N], BF16, tag="vsb")
                nc.scalar.copy(vsb[:], pv[:])
                pb = pmm.tile([128, BL * SCN * N], F32, tag="seq")
                for b in range(BL):
                    for sc in range(SCN):
                        for dc in range(DCN):
                            nc.tensor.matmul(
                                pb[:, ds((b * SCN + sc) * N, N)],
                                xts[b][:, dc, ds(sc * 128, 128)],
                                vsb[:, ds((b * DCN + dc) * N, N)],
                                start=(dc == 0),
                                stop=(dc == DCN - 1),
                            )
                return pb

            # iter 0: uniform routing weights -> m0 = xsum @ W (diag blocks)
            pot0 = pmm.tile([128, BL * KCN], F32, tag="seq")
            for b in range(BL):
                for kc in range(KCN):
                    for dc in range(DCN):
                        nc.tensor.matmul(
                            pot0[:, ds(b * KCN + kc, 1)],
                            wsb[:, dc, ds(kc * 128, 128)],
                            xsumb[:, ds(b * DCN + dc, 1)],
                            start=(dc == 0),
                            stop=(dc == DCN - 1),
                        )
            mnorm = squash(pot0, 1, 0)
            pb = v_and_b(mnorm, 0)

            for it in range(1, ROUTINGS):
                # softmax over n
                expb = prt.tile([128, BL * SCN * N], F32, tag="expb",
                                name=f"expb{it}")
                nc.scalar.activation(expb[:], pb[:], AF.Exp)
                zsum = prt.tile([128, BL * SCN], F32, tag="zsum",
                                name=f"zsum{it}")
                nc.vector.tensor_reduce(
                    zsum[:],
                    expb[:].rearrange("p (g n) -> p g n", g=BL * SCN),
                    axis=AX.X,
                    op=OP.add,
                )
                zrec = prt.tile([128, BL * SCN], F32, tag="zrec",
                                name=f"zrec{it}")
                nc.vector.reciprocal(zrec[:], zsum[:])
                cw = prt.tile([128, BL * SCN * N], BF16, tag="cw",
                              name=f"cw{it}")
                zr_bc = bass.AP(
                    tensor=zrec.tensor,
                    offset=zrec.offset,
                    ap=[zrec.ap[0], [1, BL * SCN], [0, N]],
                )
                nc.vector.tensor_mul(
                    cw[:].rearrange("p (g n) -> p g n", g=BL * SCN),
                    expb[:].rearrange("p (g n) -> p g n", g=BL * SCN),
                    zr_bc,
                )
                # G^T[d, n] per (b, dc)
                pg = pmm.tile([128, BL * DCN * N], F32, tag="big",
                              name=f"gp{it}")
                for b in range(BL):
                    for dc in range(DCN):
                        for sc in range(SCN):
                            nc.tensor.matmul(
                                pg[:, ds((b * DCN + dc) * N, N)],
                                xbs[b][:, sc, ds(dc * 128, 128)],
                                cw[:, ds((b * SCN + sc) * N, N)],
                                start=(sc == 0),
                                stop=(sc == SCN - 1),
                            )
                gsb = prt.tile([128, BL * DCN * N], BF16, tag="gsb",
                               name=f"gsb{it}")
                nc.scalar.copy(gsb[:], pg[:])
                # outT[(nc), n] per (b, kc)
                pot = pmm.tile([128, BL * KCN * N], F32, tag="seq",
                               name=f"potp{it}")
                for b in range(BL):
                    for kc in range(KCN):
                        for dc in range(DCN):
                            nc.tensor.matmul(
                                pot[:, ds((b * KCN + kc) * N, N)],
                                wsb[:, dc, ds(kc * 128, 128)],
                                gsb[:, ds((b * DCN + dc) * N, N)],
                                start=(dc == 0),
                                stop=(dc == DCN - 1),
                            )
                mnorm = squash(pot, N, it)
                if it < ROUTINGS - 1:
                    pb = v_and_b(mnorm, it)

            # final output: mnorm [128=(nl,c), (b kc)] f32 -> OUT[b, n, c]
            nc.sync.dma_start(
                OUT.rearrange("b (kc nl) c -> (nl c) (b kc)", kc=KCN, nl=4),
                mnorm[:],
            )

    nc.compile()
    return nc


def _make_consts():
    import ml_dtypes
    con = np.zeros((128, CONW), dtype=np.float32)
    con[:, CID:CID + 128] = np.eye(128, dtype=np.float32)
    p = np.arange(128)
    for b in range(BL):
        for kc in range(KCN):
            for n in range(N):
                con[:, CMASK + (b * KCN + kc) * N + n] = (n == 4 * kc + p // 32)
    for j in range(4):
        con[:, CSEL + j] = (p // 32 == j)
    con[:, CONE] = 1.0
    for j in range(4):
        con[j, CSELT:CSELT + 128] = (p // 32 == j)
    return con.astype(ml_dtypes.bfloat16)


_NC_CACHE = []


def kernel(x: np.ndarray, W: np.ndarray) -> np.ndarray:
    import ml_dtypes
    assert x.shape == (B, S, D) and W.shape == (1, D, NC)
    if not _NC_CACHE:
        _NC_CACHE.append(_build_module())
    nc = _NC_CACHE[0]
    con = _make_consts()
    w2 = np.ascontiguousarray(W[0]).astype(ml_dtypes.bfloat16)
    xb = x.astype(ml_dtypes.bfloat16)
    in_maps = []
    for i in range(NCORES):
        m = {
            "x": np.ascontiguousarray(xb[i * BL:(i + 1) * BL]),
            "w": w2,
            "consts": con,
        }
        in_maps.append(m)
    res = run_bass_kernel_spmd(nc, in_maps, list(range(NCORES)))
    out = np.concatenate([res.results[i]["out"] for i in range(NCORES)], axis=0)
    return out.astype(np.float32)
